# revision 1
# baseline (speedup 1.0000x reference)
"""MoE top-2 routing kernel for 8 Trainium2 NeuronCores.

Strategy (expert-parallel, per spec sharding hint):
  - Host computes the (cheap) gate: softmax -> top-2 -> renormalized scores.
  - Tokens are gathered per expert on the host; core e runs a dense
    gelu-MLP (fc1 -> gelu -> fc2) for expert e over its gathered tokens.
  - Device matmuls run in float32r (1 cycle/row on the PE when the moving
    dim >= 256, i.e. full speed) streaming weights from DRAM; the host
    applies the combine scores + b2 and scatter-adds the results back.

Per-core device work (C ~= 4608 padded tokens):
  fc1: h = gelu(x @ w1 + b1)   [C,1024] @ [1024,4096]
  fc2: y = h @ w2              [C,4096] @ [4096,1024]
  PE floor ~= C * 512 cycles / 2.4GHz ~= 1.0 ms; DMA ~= 320MB overlapped.
"""

import sys

sys.path.insert(0, "/opt/trn_rl_repo")

from contextlib import ExitStack

import numpy as np

from concourse import bacc, mybir, tile
from concourse.bass_utils import run_bass_kernel_spmd

E, H, I = 8, 1024, 4096
TOP_K = 2
N_CORES = 8

TT = 512  # token tile (two PSUM subtiles of 256)
SUB = TT // 2

F32 = mybir.dt.float32
F32R = mybir.dt.float32r


def _build_nc(C: int, act_func=None, repeat=1):
    """One SPMD program: dense expert MLP over C gathered tokens."""
    nc = bacc.Bacc(
        "TRN2", target_bir_lowering=False, debug=False, num_devices=N_CORES
    )
    xT = nc.dram_tensor("xT", [8, 128, C], F32R, kind="ExternalInput").ap()
    w1p = nc.dram_tensor("w1p", [8, 128, 4, 1024], F32R, kind="ExternalInput").ap()
    b1p = nc.dram_tensor("b1p", [128, 32], F32, kind="ExternalInput").ap()
    w2p = nc.dram_tensor("w2p", [8, 128, 4096], F32R, kind="ExternalInput").ap()
    yT = nc.dram_tensor("yT", [8, 128, C], F32, kind="ExternalOutput").ap()

    n_tiles = C // TT
    gelu = act_func if act_func is not None else mybir.ActivationFunctionType.Gelu

    with tile.TileContext(nc) as tc, ExitStack() as ctx:
        b1pool = ctx.enter_context(tc.tile_pool(name="b1", bufs=1))
        xpool = ctx.enter_context(tc.tile_pool(name="x", bufs=2))
        hpool = ctx.enter_context(tc.tile_pool(name="h", bufs=1))
        w1pool = ctx.enter_context(tc.tile_pool(name="w1", bufs=2))
        w2pool = ctx.enter_context(tc.tile_pool(name="w2", bufs=2))
        ypool = ctx.enter_context(tc.tile_pool(name="y", bufs=3))
        ps1 = ctx.enter_context(tc.tile_pool(name="ps1", bufs=3, space="PSUM"))
        ps2 = ctx.enter_context(tc.tile_pool(name="ps2", bufs=3, space="PSUM"))

        b1t = b1pool.tile([128, 32], F32)
        nc.sync.dma_start(out=b1t[:], in_=b1p)

        rep_ctx = tc.For_i(0, repeat, 1) if repeat > 1 else None
        if rep_ctx is not None:
            ctx.enter_context(rep_ctx)

        for t in range(n_tiles):
            t0 = t * TT
            xt = xpool.tile([128, 8, TT], F32R)
            for kc in range(8):
                nc.sync.dma_start(out=xt[:, kc, :], in_=xT[kc, :, t0 : t0 + TT])

            ht = hpool.tile([128, 32, TT], F32R)

            # fc1 + gelu: 32 output chunks of 128, contraction over 8 k-chunks
            for ms in range(8):  # w1 slabs of 4 m-chunks (2MB DMA each)
                w1t = w1pool.tile([128, 4, 1024], F32R)
                nc.gpsimd.dma_start(out=w1t[:], in_=w1p[ms])
                for j in range(4):
                    mc = ms * 4 + j
                    for sub in range(2):
                        ps = ps1.tile([128, SUB], F32)
                        for kc in range(8):
                            nc.tensor.matmul(
                                ps[:],
                                lhsT=w1t[:, j, kc * 128 : (kc + 1) * 128],
                                rhs=xt[:, kc, sub * SUB : (sub + 1) * SUB],
                                start=(kc == 0),
                                stop=(kc == 7),
                            )
                        nc.scalar.activation(
                            out=ht[:, mc, sub * SUB : (sub + 1) * SUB],
                            in_=ps[:],
                            func=gelu,
                            bias=b1t[:, mc : mc + 1],
                            scale=1.0,
                        )

            # fc2: 8 output chunks of 128, contraction over 32 i-chunks
            for m2c in range(8):
                w2t = w2pool.tile([128, 4096], F32R)
                nc.gpsimd.dma_start(out=w2t[:], in_=w2p[m2c])
                yt = ypool.tile([128, TT], F32)
                for sub in range(2):
                    ps = ps2.tile([128, SUB], F32)
                    for ic in range(32):
                        nc.tensor.matmul(
                            ps[:],
                            lhsT=w2t[:, ic * 128 : (ic + 1) * 128],
                            rhs=ht[:, ic, sub * SUB : (sub + 1) * SUB],
                            start=(ic == 0),
                            stop=(ic == 31),
                        )
                    nc.vector.tensor_copy(
                        out=yt[:, sub * SUB : (sub + 1) * SUB], in_=ps[:]
                    )
                nc.sync.dma_start(out=yT[m2c, :, t0 : t0 + TT], in_=yt[:])

    nc.compile()
    return nc


def _route(x_flat, gate_w, gate_b, alpha):
    """Host gate: returns per-expert (row_indices, combine_scores)."""
    logits = x_flat @ gate_w + gate_b
    m = logits.max(axis=-1, keepdims=True)
    p = np.exp(logits - m)
    p /= p.sum(axis=-1, keepdims=True)
    idx = np.argpartition(p, E - TOP_K, axis=-1)[:, -TOP_K:]  # top-2 (unordered)
    vals = np.take_along_axis(p, idx, axis=-1)
    sc = vals / vals.sum(axis=-1, keepdims=True)
    sc = sc * alpha[idx]
    routes = []
    for e in range(E):
        mask = idx == e  # at most one True per row (top-k distinct)
        rows = np.nonzero(mask.any(axis=1))[0]
        scores = sc[mask]  # row-major order matches `rows`
        routes.append((rows, scores.astype(np.float32)))
    return routes


def prepare(hidden_states, gate_w, gate_b, w1, b1, w2, b2, alpha):
    """Host routing + input prep. Returns (nc, in_maps, state)."""
    x = np.asarray(hidden_states, dtype=np.float32)
    gate_w = np.asarray(gate_w, dtype=np.float32)
    gate_b = np.asarray(gate_b, dtype=np.float32)
    w1 = np.asarray(w1, dtype=np.float32)
    b1 = np.asarray(b1, dtype=np.float32)
    w2 = np.asarray(w2, dtype=np.float32)
    b2 = np.asarray(b2, dtype=np.float32)
    alpha = np.asarray(alpha, dtype=np.float32)

    B, S, Hd = x.shape
    T = B * S
    xf = x.reshape(T, Hd)

    routes = _route(xf, gate_w, gate_b, alpha)
    max_cnt = max(len(r) for r, _ in routes)
    C = max(TT, ((max_cnt + TT - 1) // TT) * TT)

    nc = _build_nc(C)

    in_maps = []
    for e in range(E):
        rows, _ = routes[e]
        xTe = np.zeros((8, 128, C), dtype=np.float32)
        if len(rows):
            xTe[:, :, : len(rows)] = (
                xf[rows].T.reshape(8, 128, len(rows))
            )
        w1pe = np.ascontiguousarray(
            w1[e].reshape(8, 128, 32, 128).transpose(2, 1, 0, 3).reshape(32, 128, 1024)
            .reshape(8, 4, 128, 1024).transpose(0, 2, 1, 3)
        )
        b1pe = np.ascontiguousarray(b1[e].reshape(32, 128).T)
        w2pe = np.ascontiguousarray(
            w2[e].reshape(32, 128, 8, 128).transpose(2, 1, 0, 3).reshape(8, 128, 4096)
        )
        in_maps.append({"xT": xTe, "w1p": w1pe, "b1p": b1pe, "w2p": w2pe})

    state = dict(routes=routes, C=C, b2=b2, B=B, S=S, Hd=Hd, T=T)
    return nc, in_maps, state


def finalize(results, state):
    routes, C, b2 = state["routes"], state["C"], state["b2"]
    T, Hd = state["T"], state["Hd"]
    out = np.zeros((T, Hd), dtype=np.float32)
    for e in range(E):
        rows, scores = routes[e]
        if not len(rows):
            continue
        yTe = results[e]["yT"].reshape(1024, C)
        ye = yTe[:, : len(rows)].T  # [cnt, 1024]
        out[rows] += scores[:, None] * (ye + b2[e])
    return out.reshape(state["B"], state["S"], Hd)


def kernel(hidden_states, gate_w, gate_b, w1, b1, w2, b2, alpha):
    nc, in_maps, state = prepare(
        hidden_states, gate_w, gate_b, w1, b1, w2, b2, alpha
    )
    res = run_bass_kernel_spmd(nc, in_maps, list(range(N_CORES)))
    return finalize(res.results, state)



# revision 2
# speedup vs baseline: 1.8645x; 1.8645x over previous
"""MoE top-2 routing kernel for 8 Trainium2 NeuronCores.

Strategy (expert-parallel with I-sharding, per spec sharding hint):
  - Host computes the (cheap) gate: softmax -> top-2 -> renormalized scores.
  - Each expert's MLP is split into SPLIT=4 shards along the intermediate
    dim I; the 8 experts x 4 shards = 32 shard-tasks are placed on an
    8-core x 4-slot grid (2 experts per slot column, paired big/small by
    routed token count) so per-core work is near the 874us PE ideal.
  - Weights are bf16 and fully SBUF-resident (128 KiB/partition for both
    layers), so tokens stream while weights load once: DMA drops from
    ~320 MB/core (fp32r re-streaming baseline) to ~120 MB/core and the
    kernel is PE-bound at ~1 cycle/row bf16.
  - Host sums the 4 I-shard partials per expert, applies combine scores
    + b2, and scatter-adds into the full output.

Per-core device work (C ~= 16.4k shard-token columns, 128 PE cycles each):
  fc1 quarter: h = gelu(x @ w1q + b1q)   8x8 mm chunks per 512-col tile
  fc2 quarter: y_partial = h @ w2q       8x8 mm chunks per 512-col tile
  PE floor ~= C * 128 cycles / 2.4GHz ~= 880 us; DMA ~= 120 MB overlapped.
"""

import sys

sys.path.insert(0, "/opt/trn_rl_repo")

from contextlib import ExitStack

import numpy as np
import ml_dtypes

from concourse import bacc, mybir, tile
from concourse.bass_utils import run_bass_kernel_spmd

E, H, I = 8, 1024, 4096
TOP_K = 2
N_CORES = 8

SPLIT = 4          # I-shards per expert
NSLOT = 4          # shard slots per core (E * SPLIT / N_CORES)
IQ = I // SPLIT    # 1024 intermediate dims per shard
MC1 = IQ // 128    # fc1 output chunks per shard
KC1 = H // 128     # fc1 contraction chunks
MC2 = H // 128     # fc2 output chunks
KC2 = IQ // 128    # fc2 contraction chunks per shard

TT = 512           # token tile (one PSUM bank of fp32)

F32 = mybir.dt.float32
BF16 = mybir.dt.bfloat16
BF16_NP = ml_dtypes.bfloat16


def _token_tiles(caps):
    """Static schedule: (slot, col_offset, len) covering sum(caps) columns."""
    out = []
    off = 0
    for s, cap in enumerate(caps):
        o = 0
        while o < cap:
            ln = min(TT, cap - o)
            out.append((s, off + o, ln))
            o += ln
        off += cap
    return out


def _build_nc(caps, repeat=1):
    """One SPMD program: NSLOT resident expert-shard MLPs over token columns."""
    C = sum(caps)
    nc = bacc.Bacc(
        "TRN2", target_bir_lowering=False, debug=False, num_devices=N_CORES
    )
    xT = nc.dram_tensor("xT", [128, KC1, C], BF16, kind="ExternalInput").ap()
    w1p = nc.dram_tensor(
        "w1p", [NSLOT, 128, MC1, H], BF16, kind="ExternalInput"
    ).ap()
    w2p = nc.dram_tensor(
        "w2p", [NSLOT, 128, MC2, IQ], BF16, kind="ExternalInput"
    ).ap()
    b1p = nc.dram_tensor("b1p", [NSLOT, 128, MC1], F32, kind="ExternalInput").ap()
    yT = nc.dram_tensor("yT", [128, MC2, C], F32, kind="ExternalOutput").ap()

    gelu = mybir.ActivationFunctionType.Gelu

    with tile.TileContext(nc) as tc, ExitStack() as ctx:
        wpool = ctx.enter_context(tc.tile_pool(name="w", bufs=1))
        xpool = ctx.enter_context(tc.tile_pool(name="x", bufs=2))
        hpool = ctx.enter_context(tc.tile_pool(name="h", bufs=2))
        ypool = ctx.enter_context(tc.tile_pool(name="y", bufs=1))
        ps1 = ctx.enter_context(tc.tile_pool(name="ps1", bufs=3, space="PSUM"))
        ps2 = ctx.enter_context(tc.tile_pool(name="ps2", bufs=3, space="PSUM"))

        b1t = wpool.tile([128, NSLOT * MC1], F32)
        w1s = wpool.tile([128, NSLOT * MC1, H], BF16)
        w2s = wpool.tile([128, NSLOT * MC2, IQ], BF16)
        nc.sync.dma_start(out=b1t[:], in_=b1p[:, :, :])
        # slot-interleaved so slot 0 compute can start after ~4 MB of load
        for s in range(NSLOT):
            nc.gpsimd.dma_start(
                out=w1s[:, s * MC1 : (s + 1) * MC1, :], in_=w1p[s]
            )
            nc.gpsimd.dma_start(
                out=w2s[:, s * MC2 : (s + 1) * MC2, :], in_=w2p[s]
            )

        rep_ctx = tc.For_i(0, repeat, 1) if repeat > 1 else None
        if rep_ctx is not None:
            ctx.enter_context(rep_ctx)

        for s, t0, ln in _token_tiles(caps):
            xt = xpool.tile([128, KC1, ln], BF16, tag="xt")
            nc.sync.dma_start(out=xt[:], in_=xT[:, :, t0 : t0 + ln])

            ht = hpool.tile([128, KC2, ln], BF16, tag="ht")
            for mc in range(MC1):
                ps = ps1.tile([128, ln], F32, tag="ps1")
                for kc in range(KC1):
                    nc.tensor.matmul(
                        ps[:],
                        lhsT=w1s[:, s * MC1 + mc, kc * 128 : (kc + 1) * 128],
                        rhs=xt[:, kc, :],
                        start=(kc == 0),
                        stop=(kc == KC1 - 1),
                    )
                nc.scalar.activation(
                    out=ht[:, mc, :],
                    in_=ps[:],
                    func=gelu,
                    bias=b1t[:, s * MC1 + mc : s * MC1 + mc + 1],
                    scale=1.0,
                )

            yt = ypool.tile([128, MC2, ln], F32, tag="yt")
            for mc in range(MC2):
                ps = ps2.tile([128, ln], F32, tag="ps2")
                for ic in range(KC2):
                    nc.tensor.matmul(
                        ps[:],
                        lhsT=w2s[:, s * MC2 + mc, ic * 128 : (ic + 1) * 128],
                        rhs=ht[:, ic, :],
                        start=(ic == 0),
                        stop=(ic == KC2 - 1),
                    )
                nc.vector.tensor_copy(out=yt[:, mc, :], in_=ps[:])
            nc.sync.dma_start(out=yT[:, :, t0 : t0 + ln], in_=yt[:])

    nc.compile()
    return nc


def _route(x_flat, gate_w, gate_b, alpha):
    """Host gate: returns per-expert (row_indices, combine_scores)."""
    logits = x_flat @ gate_w + gate_b
    m = logits.max(axis=-1, keepdims=True)
    p = np.exp(logits - m)
    p /= p.sum(axis=-1, keepdims=True)
    idx = np.argpartition(p, E - TOP_K, axis=-1)[:, -TOP_K:]  # top-2 (unordered)
    vals = np.take_along_axis(p, idx, axis=-1)
    sc = vals / vals.sum(axis=-1, keepdims=True)
    sc = sc * alpha[idx]
    routes = []
    for e in range(E):
        mask = idx == e  # at most one True per row (top-k distinct)
        rows = np.nonzero(mask.any(axis=1))[0]
        scores = sc[mask]  # row-major order matches `rows`
        routes.append((rows, scores.astype(np.float32)))
    return routes


def _plan(counts):
    """Place 8 experts x SPLIT shards on the 8-core x NSLOT-slot grid.

    Slot s holds experts order[2s] (its SPLIT shards on cores 0..SPLIT-1)
    and order[2s+1] (on cores SPLIT..2*SPLIT-1); cap_s = max of the two
    counts. Sorted placement minimizes sum_s cap_s.
    """
    order = np.argsort(-np.asarray(counts), kind="stable")
    assign = [[None] * NSLOT for _ in range(N_CORES)]
    caps = []
    for s in range(NSLOT):
        ea, eb = int(order[2 * s]), int(order[2 * s + 1])
        for q in range(SPLIT):
            assign[q][s] = (ea, q)
            assign[SPLIT + q][s] = (eb, q)
        caps.append(int(max(counts[ea], counts[eb])))
    return assign, caps


def prepare(hidden_states, gate_w, gate_b, w1, b1, w2, b2, alpha):
    """Host routing + input prep. Returns (nc, in_maps, state)."""
    x = np.asarray(hidden_states, dtype=np.float32)
    gate_w = np.asarray(gate_w, dtype=np.float32)
    gate_b = np.asarray(gate_b, dtype=np.float32)
    w1 = np.asarray(w1, dtype=np.float32)
    b1 = np.asarray(b1, dtype=np.float32)
    w2 = np.asarray(w2, dtype=np.float32)
    b2 = np.asarray(b2, dtype=np.float32)
    alpha = np.asarray(alpha, dtype=np.float32)

    B, S, Hd = x.shape
    T = B * S
    xf = x.reshape(T, Hd)

    routes = _route(xf, gate_w, gate_b, alpha)
    counts = [len(r) for r, _ in routes]
    assign, caps = _plan(counts)
    C = sum(caps)
    offs = np.concatenate([[0], np.cumsum(caps)]).astype(int)

    nc = _build_nc(caps)

    # Per-expert packed tokens [128, KC1, cnt] bf16, shared by the SPLIT
    # cores that hold the expert's shards.
    xTe = {}
    for e in range(E):
        rows, _ = routes[e]
        xTe[e] = np.ascontiguousarray(
            xf[rows].astype(BF16_NP).T.reshape(KC1, 128, len(rows))
            .transpose(1, 0, 2)
        )

    def pack_w(wq):
        # [128k, kc-or-ic chunks * 128 m] from [K, M]: out[p, mc, kc*128+m]
        K, M = wq.shape
        return np.ascontiguousarray(
            wq.reshape(K // 128, 128, M // 128, 128)
            .transpose(1, 2, 0, 3)
            .reshape(128, M // 128, K)
            .astype(BF16_NP)
        )

    in_maps = []
    for c in range(N_CORES):
        xTc = np.zeros((128, KC1, C), dtype=BF16_NP)
        w1c = np.zeros((NSLOT, 128, MC1, H), dtype=BF16_NP)
        w2c = np.zeros((NSLOT, 128, MC2, IQ), dtype=BF16_NP)
        b1c = np.zeros((NSLOT, 128, MC1), dtype=np.float32)
        for s in range(NSLOT):
            e, q = assign[c][s]
            cnt = counts[e]
            xTc[:, :, offs[s] : offs[s] + cnt] = xTe[e]
            w1c[s] = pack_w(w1[e][:, q * IQ : (q + 1) * IQ])
            w2c[s] = pack_w(w2[e][q * IQ : (q + 1) * IQ, :])
            b1c[s] = b1[e][q * IQ : (q + 1) * IQ].reshape(MC1, 128).T
        in_maps.append({"xT": xTc, "w1p": w1c, "w2p": w2c, "b1p": b1c})

    state = dict(
        routes=routes, counts=counts, assign=assign, caps=caps, offs=offs,
        C=C, b2=b2, B=B, S=S, Hd=Hd, T=T,
    )
    return nc, in_maps, state


def finalize(results, state):
    routes, counts = state["routes"], state["counts"]
    assign, offs = state["assign"], state["offs"]
    b2 = state["b2"]
    T, Hd = state["T"], state["Hd"]
    C = state["C"]

    # Sum the SPLIT I-shard partials per expert: [Hd, cnt] each.
    ysum = {}
    for c in range(N_CORES):
        yTc = results[c]["yT"].transpose(1, 0, 2).reshape(Hd, C)
        for s in range(NSLOT):
            e, _q = assign[c][s]
            part = yTc[:, offs[s] : offs[s] + counts[e]]
            ysum[e] = part if e not in ysum else ysum[e] + part

    out = np.zeros((T, Hd), dtype=np.float32)
    for e in range(E):
        rows, scores = routes[e]
        if not len(rows):
            continue
        out[rows] += scores[:, None] * (ysum[e].T + b2[e])
    return out.reshape(state["B"], state["S"], Hd)


def kernel(hidden_states, gate_w, gate_b, w1, b1, w2, b2, alpha):
    nc, in_maps, state = prepare(
        hidden_states, gate_w, gate_b, w1, b1, w2, b2, alpha
    )
    res = run_bass_kernel_spmd(nc, in_maps, list(range(N_CORES)))
    return finalize(res.results, state)
